# revision 10
# baseline (speedup 1.0000x reference)
"""MHA + RoPE on 8 trn2 cores, v5 (f32r): software-pipelined Q projection.

Sharding: core c -> batch c//2, head-group c%2 (8 heads / 512 features).

Schedule per core:
  head: K projection+RoPE (all S), V projection, Q projection+RoPE for
        q-chunks 0,1. ACT does the K/Q bias adds (otherwise idle here).
  body: per q-chunk qc: scores+exp+PV per head-pair (v1 pipeline); the Q
        projection for qc+2 is interleaved into the pair loop (matmuls at
        g==1, RoPE at g==3) sharing the 1KB "pv" PSUM ring, so the PE does
        it inside the ACT-paced exp pipeline instead of a serial phase.
"""

import contextlib

import numpy as np

import concourse.bass as bass
import concourse.mybir as mybir
import concourse.tile as tile
from concourse import bacc
from concourse.bass_utils import run_bass_kernel_spmd

F32 = mybir.dt.float32
F32R = mybir.dt.float32r
AF = mybir.ActivationFunctionType
ADD = mybir.AluOpType.add
MULT = mybir.AluOpType.mult

B, S, D, H = 4, 2048, 1024, 16
HD = D // H            # 64
NCORES = 8
FC = D // 2
NH = FC // HD          # 8
QN = 256
XS = 256
NQC = S // QN          # 8
NKC = S // 128         # 16
NDC = D // 128         # 8
NFC = FC // 128        # 4
NG = 4
EXP_BIAS = -8.0
SCALE = 1.0 / np.sqrt(HD)


def build_kernel(dump=False, repeat=1):
    nc = bacc.Bacc("TRN2", debug=False)

    xp = nc.dram_tensor("xp", [S // XS, 128, NDC, XS], F32R, kind="ExternalInput")
    wq = nc.dram_tensor("wq", [128, NDC, FC], F32R, kind="ExternalInput")
    wk = nc.dram_tensor("wk", [128, NDC, FC], F32R, kind="ExternalInput")
    wv = nc.dram_tensor("wv", [128, NDC, FC], F32R, kind="ExternalInput")
    wo = nc.dram_tensor("wo", [128, NFC, D], F32R, kind="ExternalInput")
    bq = nc.dram_tensor("bq", [FC], F32, kind="ExternalInput")
    bk = nc.dram_tensor("bk", [FC], F32, kind="ExternalInput")
    bv = nc.dram_tensor("bv", [1, FC], F32R, kind="ExternalInput")
    c2 = nc.dram_tensor("c2", [128, S], F32, kind="ExternalInput")
    s2 = nc.dram_tensor("s2", [128, S], F32, kind="ExternalInput")
    onesin = nc.dram_tensor("onesin", [1, S], F32R, kind="ExternalInput")
    perm = nc.dram_tensor("perm", [128, 128], F32R, kind="ExternalInput")
    yT = nc.dram_tensor("yT", [D, S], F32, kind="ExternalOutput")
    if dump:
        qT_d = nc.dram_tensor("qT_d", [FC, S], F32, kind="ExternalOutput")
        kT_d = nc.dram_tensor("kT_d", [FC, S], F32, kind="ExternalOutput")
        vt_d = nc.dram_tensor("vt_d", [S, NH, HD + 1], F32, kind="ExternalOutput")

    with tile.TileContext(nc) as tc:
      for _rep in range(repeat):
       with contextlib.ExitStack() as ctx:
        ll = ctx.enter_context(tc.tile_pool(name="ll", bufs=1))

        kT = [ll.tile([128, S], F32R, name=f"kT{i}") for i in range(NFC)]
        vt = [ll.tile([128, NH, HD + 1], F32R, name=f"vt{k}") for k in range(NKC)]
        ebias = ll.tile([128, 1], F32, name="ebias")
        ones_sb = ll.tile([1, S], F32R, name="ones_sb")
        ones_col = ll.tile([128, NH], F32R, name="ones_col")
        perm_sb = ll.tile([128, 128], F32R, name="perm_sb")
        bqs = ll.tile([128, NFC], F32, name="bqs")
        bks = ll.tile([128, NFC], F32, name="bks")
        pq = ctx.enter_context(tc.tile_pool(name="pq", bufs=1))
        q_tiles = {}

        def new_q_tile(qcn, fc):
            t = pq.tile([128, XS], F32R, name="qt", tag=f"qt{fc}", bufs=3)
            q_tiles[(qcn, fc)] = t
            return t

        nc.vector.memset(ebias, EXP_BIAS)
        nc.sync.dma_start(out=ones_sb, in_=onesin[:])
        ones_dram = onesin[:]
        nc.sync.dma_start(
            out=ones_col,
            in_=bass.AP(tensor=ones_dram.tensor, offset=ones_dram.offset,
                        ap=[[0, 128], [1, NH]]))
        nc.sync.dma_start(out=perm_sb, in_=perm[:])
        nc.sync.dma_start(out=bqs, in_=bq[:].rearrange("(c p) -> p c", p=128))
        nc.sync.dma_start(out=bks, in_=bk[:].rearrange("(c p) -> p c", p=128))

        xp_r = xp[:]

        # ---- head: K (all S) + V (all S) + Q (q-chunks 0,1), one xp pass ----
        with tc.tile_pool(name="pA", bufs=1) as pA, \
             tc.tile_pool(name="ppA", bufs=2, space="PSUM") as ppA:
            wk_sb = pA.tile([128, NDC, FC], F32R, name="wk_sb")
            wv_sb = pA.tile([128, NDC, FC], F32R, name="wv_sb")
            wqh_sb = pA.tile([128, NDC, FC], F32R, name="wqh_sb")
            bv_sb = pA.tile([1, FC], F32R, name="bv_sb")
            c2_sb = pA.tile([128, S], F32, name="c2_sb")
            s2_sb = pA.tile([128, S], F32, name="s2_sb")
            nc.sync.dma_start(out=bv_sb, in_=bv[:])
            nc.sync.dma_start(out=wk_sb, in_=wk[:])
            nc.sync.dma_start(out=wv_sb, in_=wv[:])
            nc.sync.dma_start(out=wqh_sb, in_=wq[:])
            nc.sync.dma_start(out=c2_sb, in_=c2[:])
            nc.sync.dma_start(out=s2_sb, in_=s2[:])

            def proj_rope_head(w_sb, bias_t, out_ap, xh, sg):
                ps = ppA.tile([128, XS], F32, name="ps", tag="proj", bufs=3)
                for d in range(NDC):
                    nc.tensor.matmul(
                        ps, w_sb[:, d, :], xh[:, d, :],
                        start=(d == 0), stop=(d == NDC - 1),
                    )
                praw = pA.tile([128, XS], F32R, name="praw", tag="praw", bufs=3)
                nc.scalar.add(praw, ps, bias_t)
                swps = ppA.tile([128, XS], F32, name="swps", tag="swp", bufs=2)
                nc.tensor.matmul(swps, perm_sb, praw, start=True, stop=True)
                prod = pA.tile([128, XS], F32, name="prod", tag="prod", bufs=3)
                nc.vector.tensor_mul(prod, swps, s2_sb[:, sg:sg + XS])
                nc.vector.tensor_mul(out_ap, praw, c2_sb[:, sg:sg + XS])
                nc.gpsimd.tensor_add(out_ap, out_ap, prod)

            for scg in range(S // XS):
                xh = pA.tile([128, NDC, XS], F32R, name="xh", tag="xh", bufs=2)
                nc.sync.dma_start(out=xh, in_=xp_r[scg])
                sg = scg * XS
                for fc in range(NFC):
                    proj_rope_head(
                        wk_sb[:, :, fc * 128:(fc + 1) * 128], bks[:, fc:fc + 1],
                        kT[fc][:, sg:sg + XS], xh, sg)
                if scg < 2:
                    for fc in range(NFC):
                        proj_rope_head(
                            wqh_sb[:, :, fc * 128:(fc + 1) * 128],
                            bqs[:, fc:fc + 1],
                            new_q_tile(scg, fc), xh, sg)
                for ss in range(XS // 128):
                    kg = scg * (XS // 128) + ss
                    psv = ppA.tile([128, FC], F32, name="psv", tag="projv")
                    for d in range(NDC):
                        nc.tensor.matmul(
                            psv,
                            xh[:, d, ss * 128:(ss + 1) * 128],
                            wv_sb[:, d, :],
                            start=(d == 0),
                            stop=False,
                        )
                    nc.tensor.matmul(
                        psv, ones_sb[0:1, 0:128], bv_sb, start=False, stop=True,
                    )
                    nc.vector.tensor_copy(
                        vt[kg][:, :, 0:HD],
                        psv.rearrange("p (h e) -> p h e", e=HD),
                    )
                    nc.vector.tensor_copy(
                        vt[kg][:, :, HD:HD + 1],
                        ones_col.rearrange("p (h o) -> p h o", o=1),
                    )

        if dump:
            with tc.tile_pool(name="pD", bufs=1) as pD:
                kd_r = kT_d[:].rearrange("(c p) s -> c p s", p=128)
                for fc in range(NFC):
                    tk = pD.tile([128, S], F32, name="tk", tag="dk")
                    nc.vector.tensor_copy(tk, kT[fc])
                    nc.sync.dma_start(out=kd_r[fc], in_=tk)
                for kg in range(NKC):
                    tv = pD.tile([128, NH, HD + 1], F32, name="tv", tag="dv")
                    nc.vector.tensor_copy(tv, vt[kg])
                    nc.sync.dma_start(
                        out=vt_d[:].rearrange("(c p) h e -> c p h e", p=128)[kg],
                        in_=tv,
                    )

        # ---- body: attention; Q(qc+2) interleaved into the pair loop ----
        with tc.tile_pool(name="p3", bufs=1) as p3, \
             tc.tile_pool(name="ps_sc", bufs=2, space="PSUM") as ps_sc, \
             tc.tile_pool(name="ps_pv", bufs=2, space="PSUM") as ps_pv, \
             tc.tile_pool(name="ps_o", bufs=2, space="PSUM") as ps_o:
            wo_sb = p3.tile([128, NFC, D], F32R, name="wo_sb")
            nc.sync.dma_start(out=wo_sb, in_=wo[:])

            xq = [None, None]
            qpraw = [None]
            wqs = {}
            ctab = [None, None]

            def q_wq_prefetch(fc):
                w = p3.tile([128, NDC, 128], F32R, name="wqs", tag="wqs",
                            bufs=2)
                nc.sync.dma_start(
                    out=w, in_=wq[:, :, fc * 128:(fc + 1) * 128])
                wqs[fc] = w

            def q_mm(qcn, fc):
                ps = ps_pv.tile([128, XS], F32, name="pv_ps", tag="pv")
                for d in range(NDC):
                    nc.tensor.matmul(
                        ps, wqs[fc][:, d, :],
                        xq[qcn % 2][:, d, :],
                        start=(d == 0), stop=(d == NDC - 1),
                    )
                praw = p3.tile([128, XS], F32R, name="prawq", tag="prawq",
                               bufs=2)
                nc.vector.tensor_scalar(
                    praw, ps, bqs[:, fc:fc + 1], None, op0=ADD)
                qpraw[0] = praw

            def q_rope(qcn, fc):
                praw = qpraw[0]
                qt = new_q_tile(qcn, fc)
                swps = ps_pv.tile([128, XS], F32, name="pv_ps", tag="pv")
                nc.tensor.matmul(swps, perm_sb, praw, start=True, stop=True)
                prod = p3.tile([128, XS], F32, name="prodq", tag="prodq",
                               bufs=2)
                nc.vector.tensor_mul(prod, swps, ctab[1])
                nc.vector.tensor_mul(qt, praw, ctab[0])
                nc.gpsimd.tensor_add(qt, qt, prod)

            for qc in range(NQC):
                q_sl = slice(qc * QN, (qc + 1) * QN)
                do_q = qc <= NQC - 3
                if do_q:
                    t = p3.tile([128, NDC, XS], F32R, name="xq", tag="xq",
                                bufs=2)
                    nc.sync.dma_start(out=t, in_=xp_r[qc + 2])
                    xq[qc % 2] = t
                    sgq = (qc + 2) * XS
                    tc2 = p3.tile([128, XS], F32, name="c2s", tag="c2s",
                                  bufs=2)
                    nc.sync.dma_start(out=tc2, in_=c2[:, sgq:sgq + XS])
                    ts2 = p3.tile([128, XS], F32, name="s2s", tag="s2s",
                                  bufs=2)
                    nc.sync.dma_start(out=ts2, in_=s2[:, sgq:sgq + XS])
                    ctab[0], ctab[1] = tc2, ts2
                    q_wq_prefetch(0)

                ctxT = []
                for pair in range(NFC):
                    exp_sl = [[None] * NG for _ in range(2)]
                    for g in range(NG):
                        sc_ps = [
                            ps_sc.tile([128, NG, QN], F32, name="sc_ps",
                                       tag="sc")
                            for h in range(2)
                        ]
                        for j in range(NG):
                            kc = g * NG + j
                            k_sl = slice(kc * 128, (kc + 1) * 128)
                            for h in range(2):
                                nc.tensor.matmul(
                                    sc_ps[h][:, j, :],
                                    kT[pair][h * 64:(h + 1) * 64, k_sl],
                                    q_tiles[(qc, pair)][h * 64:(h + 1) * 64, :],
                                    start=True, stop=True,
                                    tile_position=(h * 64, 0),
                                )
                        if do_q and g == 1:
                            q_mm(qc + 2, pair)
                        if do_q and g == 2 and pair < NFC - 1:
                            q_wq_prefetch(pair + 1)
                        if do_q and g == 3:
                            q_rope(qc + 2, pair)
                        for h in range(2):
                            es = p3.tile(
                                [128, NG, QN], F32R, name="es",
                                tag=f"exp{h}{g}", bufs=2,
                            )
                            nc.scalar.activation(
                                es, sc_ps[h], AF.Exp, bias=ebias, scale=SCALE,
                            )
                            exp_sl[h][g] = es
                    for h in range(2):
                        hh = pair * 2 + h
                        pv_ps = ps_pv.tile([128, QN], F32, name="pv_ps",
                                           tag="pv")
                        for kc in range(NKC):
                            nc.tensor.matmul(
                                pv_ps[0:HD + 1, :],
                                vt[kc][:, hh, :],
                                exp_sl[h][kc // NG][:, kc % NG, :],
                                start=(kc == 0),
                                stop=(kc == NKC - 1),
                            )
                        denr = p3.tile([1, QN], F32, name="denr", tag="denr")
                        nc.vector.reciprocal(denr, pv_ps[HD:HD + 1, :])
                        denb = p3.tile([64, QN], F32, name="denb", tag="denb")
                        nc.gpsimd.partition_broadcast(denb, denr)
                        if h == 0:
                            ct = p3.tile([128, QN], F32R, name="ct",
                                         tag=f"ctx{pair}", bufs=1)
                            ctxT.append(ct)
                        nc.vector.tensor_tensor(
                            ctxT[pair][h * 64:(h + 1) * 64, :],
                            pv_ps[0:HD, :], denb, op=MULT,
                        )
                if dump:
                    qd_r = qT_d[:].rearrange("(c p) s -> c p s", p=128)
                    for fc in range(NFC):
                        tmpq = p3.tile([128, QN], F32, name="qcvt", tag="qcvt")
                        nc.vector.tensor_copy(tmpq, q_tiles[(qc, fc)])
                        nc.sync.dma_start(out=qd_r[fc][:, q_sl], in_=tmpq)
                for ec in range(NDC):
                    ops = ps_o.tile([128, QN], F32, name="ops", tag="out")
                    for fc in range(NFC):
                        nc.tensor.matmul(
                            ops,
                            wo_sb[:, fc, ec * 128:(ec + 1) * 128],
                            ctxT[fc],
                            start=(fc == 0),
                            stop=(fc == NFC - 1),
                        )
                    ysb = p3.tile([128, QN], F32, name="ysb", tag="y", bufs=3)
                    nc.vector.tensor_copy(ysb, ops)
                    nc.sync.dma_start(
                        out=yT[:].rearrange("(c p) s -> c p s", p=128)[
                            ec, :, q_sl],
                        in_=ysb,
                    )

    nc.finalize()
    return nc


def _rope_tables():
    inv_freq = 1.0 / (10000.0 ** (np.arange(0, HD, 2, dtype=np.float64) / HD))
    pos = np.arange(S, dtype=np.float64)
    sinu = pos[None, :] * inv_freq[:, None]          # [32, S]
    c = np.sin(sinu).astype(np.float32)              # torch code calls this 'cos'
    s = np.cos(sinu).astype(np.float32)              # and this 'sin'
    c2 = np.tile(c, (4, 1))                          # [128, S]
    s2 = np.concatenate([-s, s, -s, s], axis=0)      # [128, S]
    return np.ascontiguousarray(c2), np.ascontiguousarray(s2)


def make_in_maps(inp):
    c2, s2 = _rope_tables()
    ones = np.ones((1, S), np.float32)
    pm = np.zeros((128, 128), np.float32)
    for h in range(2):
        for j in range(32):
            pm[h * 64 + 32 + j, h * 64 + j] = 1.0
            pm[h * 64 + j, h * 64 + 32 + j] = 1.0
    maps = []
    for c in range(NCORES):
        b, hg = c // 2, c % 2
        fsl = slice(hg * FC, (hg + 1) * FC)
        x = np.asarray(inp["hidden_states"][b], np.float32)
        xp = np.ascontiguousarray(
            x.reshape(S // XS, XS, NDC, 128).transpose(0, 3, 2, 1))
        wqp = np.ascontiguousarray(
            np.asarray(inp["Wq"], np.float32)[fsl].T.reshape(NDC, 128, FC)
            .transpose(1, 0, 2))
        wkp = np.ascontiguousarray(
            np.asarray(inp["Wk"], np.float32)[fsl].T.reshape(NDC, 128, FC)
            .transpose(1, 0, 2))
        wvp = np.ascontiguousarray(
            np.asarray(inp["Wv"], np.float32)[fsl].T.reshape(NDC, 128, FC)
            .transpose(1, 0, 2))
        wop = np.ascontiguousarray(
            np.asarray(inp["Wo"], np.float32)[:, fsl].T.reshape(NFC, 128, D)
            .transpose(1, 0, 2))
        maps.append({
            "xp": xp, "wq": wqp, "wk": wkp, "wv": wvp, "wo": wop,
            "bq": np.ascontiguousarray(np.asarray(inp["bq"], np.float32)[fsl]),
            "bk": np.ascontiguousarray(np.asarray(inp["bk"], np.float32)[fsl]),
            "bv": np.ascontiguousarray(
                np.asarray(inp["bv"], np.float32)[fsl][None, :]),
            "c2": c2, "s2": s2, "onesin": ones, "perm": pm,
        })
    return maps


_NC_CACHE = {}


def kernel(hidden_states, Wq, bq, Wk, bk, Wv, bv, Wo, bo):
    if "nc" not in _NC_CACHE:
        _NC_CACHE["nc"] = build_kernel()
    nc = _NC_CACHE["nc"]
    in_maps = make_in_maps({
        "hidden_states": hidden_states, "Wq": Wq, "bq": bq, "Wk": Wk, "bk": bk,
        "Wv": Wv, "bv": bv, "Wo": Wo,
    })
    res = run_bass_kernel_spmd(nc, in_maps, list(range(NCORES)))
    bo = np.asarray(bo, np.float32)
    out = np.empty((B, S, D), np.float32)
    for b in range(B):
        acc = res.results[2 * b]["yT"] + res.results[2 * b + 1]["yT"]
        out[b] = acc.T + bo[None, :]
    return out
